# revision 55
# baseline (speedup 1.0000x reference)
"""Bahdanau-attention alignment model on 8 TRN2 NeuronCores (fp8 DoubleRow).

Math (per batch b):
    wq  = dh[b] @ W_w.T + W_b                      [H]
    uk  = enc[b] @ U_w.T + U_b                     [S, H]
    act = tanh(uk + wq)                            [S, H]
    s   = act @ V_w[0]          (+V_b, dropped: softmax-invariant)
    w   = softmax(s)                               [S]
    ctx = w @ enc[b]                               [2H]

Sharding: data-parallel over batch (32 -> 4 per core), params replicated.

Per-core kernel, pipelined at s-tile-PAIR granularity:
  - The dominant U matmul runs in fp8 (e4m3) with perf_mode=DoubleRow:
    2 fp8 weights per PE cell -> K=256 contraction per instruction, halving
    the matmul count vs bf16. enc is quantized to e4m3 (x16 scale) and U_w
    to e4m3 (x256 scale) on the host; the 1/4096 descale folds into the
    tanh's scale operand (ScalarE computes func(in*scale + bias)).
  - All layouts are prepared host-side: encT8 arrives pre-transposed in the
    DoubleRow [128, d_subtile, s] layout (plain HWDGE loads, no xbar
    transposes), enc natural-layout rows arrive in bf16 for pass 2.
  - Two s-tiles (A/B) are processed per weight pass, with A/B matmuls
    interleaved per d-tile so each U weight tile is loaded once per pair:
    halves LDWEIGHTS traffic and the per-accumulation-group start taxes.
    (The first pair runs A-then-B de-interleaved so the B tile's DMA
    cannot head-of-line block the A stream during startup.)
  - tanh writes act in e4m3 (j-subtile pairs packed [128, 2, 512]), so the
    V dot-product also runs as DoubleRow fp8, batched as one 8-MM block per
    pair (fired at the next pair's start) so the U weight pipeline breaks
    only twice per pair. The last pair issues them incrementally instead,
    to shorten the un-overlapped final tail.
  - wq chunk j is emitted BEFORE U group j during the first pair, so the
    tanh chain never gates the uk bank rotation; each chunk gates only on
    its own 256 KB WwT slice.
  - softmax uses a fixed offset M0 = sum|V8|/256 >= max|score| instead of
    the data max (exactly equivalent after normalization), so exp runs
    per s-tile straight out of PSUM (accum_out provides the partial sum)
    and pass 2 pipelines with pass 1 instead of waiting for all scores.
  - e is transposed to eT[128, 1] columns via tiny K=1 matmuls against a
    constant ones[1,1]; pass 2 streams enc in natural layout [s=128, d]
    and accumulates ctx = e @ enc into a single PSUM bank: the four
    512-wide d-range groups are packed at base partitions 0/32/64/96 via
    tile_position col-tiling. DVE normalizes by 1/sum(e) at the end.
  - Startup DMA discipline (the fabric is saturated for the first ~25us,
    and per-ring throughput under contention is roughly constant, so the
    ~12 MB of early traffic is spread across all three rings): encT8 owns
    the sync ring, the wq chain (dhT/bias/WwT per-j chunks) owns the
    scalar ring, UwT8 per-j chunks + the encN stream own the SWDGE ring,
    each FIFO-ordered by consumption. Small params are padded to >=1KB
    per partition (64B-per-partition DMAs are descriptor-dominated and
    their completion semaphores trickle in ~15us late, gating whatever
    PE split-wait carrier references them); -M0 rides inside the bias
    tensor for the same reason. A short block of full-array dummy matmuls
    warms the PE clock gate (HAM) during the initial DMA wait.
"""

import numpy as np
import ml_dtypes

import concourse.bass as bass
import concourse.mybir as mybir
import concourse.tile as tile
from concourse.bass_utils import run_bass_kernel_spmd

F32 = mybir.dt.float32
BF16 = mybir.dt.bfloat16
FP8 = mybir.dt.float8e4
AF = mybir.ActivationFunctionType
PM = mybir.MatmulPerfMode

N_CORES = 8
B, S, D, H = 32, 2048, 2048, 1024
BL = B // N_CORES          # batches per core = 4
S_TILE = 512
N_ST = S // S_TILE         # 4 s-tiles per batch
N_SP = N_ST // 2           # 2 s-tile pairs per batch
KSUB = H // 128            # 8 k subtiles
KP = KSUB // 2             # 4 k-subtile pairs (V DoubleRow)
DT = D // 128              # 16 d subtiles of 128 (partition dim of encT8)
DT2 = DT // 2              # 8 DoubleRow d-tiles of 256
HT = H // 128              # 8 h tiles (W matmul contraction)
N_SROW = S // 128          # 16 s-row tiles per batch (pass 2)

ENC_SCALE = 16.0           # enc -> e4m3 scale (N(0,1) values)
U_SCALE = 256.0            # U_w -> e4m3 scale (|U| <= 1/sqrt(2048))
V_SCALE = 256.0            # V_w -> e4m3 scale (|V| <= 1/32)
INV_UK = 1.0 / (ENC_SCALE * U_SCALE)
INV_V = 1.0 / V_SCALE


def _split_sync_waits(nc):
    """walrus in this toolchain caps sync-wait commands per instruction (1 for
    DMA, 2 for CTRL). Move excess waits onto engine-local no-op carriers that
    precede the instruction; engine streams execute in order so gating is
    identical."""
    for fn in nc.m.functions:
        for blk in fn.blocks:
            insts = blk.instructions
            new_list = []
            changed = False
            for inst in insts:
                si = inst.sync_info
                waits = list(si.on_wait) if (si and si.on_wait) else []
                if len(waits) > 1:
                    for w in waits[:-1]:
                        nop = mybir.InstNoOp(name=f"I-ws{nc.next_id()}", ins=[], outs=[])
                        nop.engine = inst.engine
                        nop.sync_info = mybir.SyncInfo(on_wait=[w], on_update=[])
                        new_list.append(nop)
                    si.on_wait = waits[-1:]
                    changed = True
                new_list.append(inst)
            if changed:
                blk.instructions = new_list


def build_nc():
    nc = bass.Bass()

    encT8 = nc.declare_dram_parameter("encT8", [BL * N_ST, 128, DT, S_TILE], FP8, isOutput=False)
    enc = nc.declare_dram_parameter("enc", [BL, S, D], BF16, isOutput=False)
    dhT = nc.declare_dram_parameter("dhT", [128, 1024], BF16, isOutput=False)
    WwT = nc.declare_dram_parameter("WwT", [128, KSUB, HT, 128], BF16, isOutput=False)
    UwT8 = nc.declare_dram_parameter("UwT8", [128, KSUB, DT, 128], FP8, isOutput=False)
    Vw8 = nc.declare_dram_parameter("Vw8", [128, KSUB, 128], FP8, isOutput=False)
    # bias_ext: cols 0..7 = per-j (W_b + U_b) lanes, col 8 = -M0 (the fixed
    # softmax offset; packed here so no tiny DMA is needed for it)
    bias = nc.declare_dram_parameter("bias", [128, 512], F32, isOutput=False)
    out = nc.declare_dram_parameter("out", [BL, D], F32, isOutput=True)

    with tile.TileContext(nc) as tc:
        with (
            tc.tile_pool(name="const", bufs=1) as const_pool,
            tc.tile_pool(name="enct", bufs=1) as enct_pool,
            tc.tile_pool(name="acts", bufs=1) as act_pool,
            tc.tile_pool(name="encn", bufs=1) as encn_pool,
            tc.tile_pool(name="smallsb", bufs=1) as small_pool,
            tc.tile_pool(name="ukps", bufs=1, space="PSUM") as uk_pool,
            tc.tile_pool(name="scps", bufs=1, space="PSUM") as sc_pool,
            tc.tile_pool(name="etps", bufs=1, space="PSUM") as et_pool,
            tc.tile_pool(name="ctxps", bufs=1, space="PSUM") as ctx_pool,
        ):
            # ---- HAM pre-warm: full-array dummy matmuls (gated only on DVE
            # memsets) keep the PE busy through the startup DMA wait so the
            # clock gate is at 8/8 when the real stream begins (M=1 matmuls
            # don't register as PE-busy for HAM; these must be 128x128)
            warm_w = const_pool.tile([128, 128], BF16, tag="warmw")
            nc.vector.memset(warm_w[:], 0.0)
            warm_rhs = const_pool.tile([128, S_TILE], BF16, tag="warm")
            nc.vector.memset(warm_rhs[:], 0.0)
            ones_bf = const_pool.tile([1, 1], BF16, tag="ones")
            nc.vector.memset(ones_bf[:], 1.0)
            # two banks, alternated: back-to-back start/stop groups on ONE
            # bank serialize at isolated-matmul latency (~1us each); across
            # two banks they pipeline at stream rate
            warm_ps = sc_pool.tile([128, S_TILE], F32, tag="sc", bufs=2, name="warmps")
            warm_ps2 = sc_pool.tile([128, S_TILE], F32, tag="sc", bufs=2, name="warmps2")
            for i in range(28):
                nc.tensor.matmul(
                    (warm_ps if i % 2 == 0 else warm_ps2)[:],
                    warm_w[:], warm_rhs[:], start=True, stop=True,
                )

            # ---- encT8 loads (1 MB per s-tile) ----
            enc_tiles = {}

            def issue_enct(b, st, split=False, eng=None):
                eng = eng or nc.sync
                encT_t = enct_pool.tile(
                    [128, DT, S_TILE], FP8, tag="encT", bufs=4, name="encTt"
                )
                if split:
                    # halve the first tile's DMA so the first U matmuls (which
                    # read d-planes 0..1) start after 0.5 MB instead of 1 MB
                    h = DT // 2
                    eng.dma_start(out=encT_t[:, :h], in_=encT8[b * N_ST + st, :, :h])
                    eng.dma_start(out=encT_t[:, h:], in_=encT8[b * N_ST + st, :, h:])
                else:
                    eng.dma_start(out=encT_t[:], in_=encT8[b * N_ST + st])
                enc_tiles[(b, st)] = encT_t

            # ---- params. Per-ring throughput under startup contention is
            # roughly constant, so spread the ~12 MB of early traffic across
            # ALL THREE rings: encT8 owns sync, the wq chain (dhT/bias/WwT
            # chunks) owns the scalar ring, UwT8 chunks + the encN stream
            # own the SWDGE ring. dhT/bias/V8 are padded to >=1KB per
            # partition: 64B-per-partition DMAs are descriptor-dominated and
            # their completion semaphores trickle in ~15us late under
            # contention, gating the wq chain.
            issue_enct(0, 0, split=True)
            issue_enct(0, 1)
            dhT_s = const_pool.tile([128, 1024], BF16, tag="dhT")
            nc.scalar.dma_start(out=dhT_s[:], in_=dhT[:])
            bias_s = const_pool.tile([128, 512], F32, tag="bias")
            nc.scalar.dma_start(out=bias_s[:], in_=bias[:])
            WwT_s = const_pool.tile([128, KSUB, HT, 128], BF16, tag="WwT")
            for j in range(KSUB):
                nc.scalar.dma_start(out=WwT_s[:, j], in_=WwT[:, j])
            V8_s = const_pool.tile([128, KSUB, 128], FP8, tag="Vw8")
            nc.scalar.dma_start(out=V8_s[:], in_=Vw8[:])
            negm0_c = bias_s[0:1, 8:9]
            UwT8_s = const_pool.tile([128, KSUB, DT, 128], FP8, tag="UwT8")
            for j in range(KSUB):
                nc.gpsimd.dma_start(out=UwT8_s[:, j], in_=UwT8[:, j])
            ones128 = const_pool.tile([1, 128], F32, tag="ones128")
            nc.vector.memset(ones128[:], 1.0)
            wqb = const_pool.tile([128, KSUB * BL], F32, tag="wqb")

            # ---- wq = dh @ W_w.T (+ W_b + U_b folded in) ----
            def emit_wq(j):
                wq_ps = uk_pool.tile([128, BL], F32, tag="uk", bufs=4, name="wqps")
                for i in range(HT):
                    nc.tensor.matmul(
                        wq_ps[:],
                        WwT_s[:, j, i, :],
                        dhT_s[:, i * BL : (i + 1) * BL],
                        start=(i == 0),
                        stop=(i == HT - 1),
                    )
                nc.vector.tensor_scalar_add(
                    wqb[:, j * BL : (j + 1) * BL], wq_ps[:], bias_s[:, j : j + 1]
                )

            # ---- main pipeline ----
            # eT/ctx work for s-tile pair sp is emitted after the U matmuls
            # of pair sp+1 so the exp -> transpose chain never stalls PE.
            batch_state = {}
            pending = []
            carry_v = [None]

            def emit_pending():
                for fn in pending:
                    fn()
                pending.clear()

            def make_tail(b, st, sc_ps, encNs):
                bs = batch_state[b]
                et_ps, ctx_ps, eT_b, esum_b = bs

                def tail():
                    e_st = small_pool.tile(
                        [1, S_TILE], BF16, tag="e", bufs=4, name="est"
                    )
                    nc.scalar.activation(
                        e_st[:],
                        sc_ps[0:1, :],
                        AF.Exp,
                        bias=negm0_c,
                        scale=INV_V,
                        accum_out=esum_b[:, st : st + 1],
                    )
                    for c in range(4):
                        nc.tensor.matmul(
                            et_ps[:, st * 4 + c : st * 4 + c + 1],
                            e_st[:, c * 128 : (c + 1) * 128],
                            ones_bf[:],
                            start=True,
                            stop=True,
                        )
                    nc.scalar.copy(
                        eT_b[:, st * 4 : (st + 1) * 4],
                        et_ps[:, st * 4 : (st + 1) * 4],
                    )
                    for i, r in enumerate(range(st * 4, (st + 1) * 4)):
                        encN = encNs[i]
                        for jj in range(4):
                            nc.tensor.matmul(
                                ctx_ps[32 * jj : 32 * jj + 1, :],
                                eT_b[:, r : r + 1],
                                encN[:, jj * 512 : (jj + 1) * 512],
                                start=(r == 0),
                                stop=(r == N_SROW - 1),
                                tile_position=(0, 32 * jj),
                            )

                return tail

            def make_epilogue(b):
                bs = batch_state[b]
                et_ps, ctx_ps, eT_b, esum_b = bs

                def epi():
                    esum_t = small_pool.tile(
                        [1, 1], F32, tag="esumt", bufs=2, name=f"esumt{b}"
                    )
                    nc.vector.tensor_reduce(
                        esum_t[:], esum_b[:], axis=mybir.AxisListType.X,
                        op=mybir.AluOpType.add,
                    )
                    rsum = small_pool.tile(
                        [1, 1], F32, tag="rsum", bufs=2, name=f"rsum{b}"
                    )
                    nc.vector.reciprocal(rsum[:], esum_t[:])
                    # per-partition scalar operands index by absolute lane:
                    # replicate 1/sum to all 128 partitions via a K=1 matmul
                    # against ones[128] before using it in the scaled copies.
                    rsum_ps = et_ps  # reuse the per-b et bank's last column
                    nc.tensor.matmul(
                        rsum_ps[:, N_SROW - 1 : N_SROW],
                        ones128[:],
                        rsum[:, 0:1],
                        start=True,
                        stop=True,
                        skip_group_check=True,
                    )
                    rsum_all = small_pool.tile(
                        [128, 1], F32, tag="rsum_all", bufs=2, name=f"rsumall{b}"
                    )
                    nc.vector.tensor_copy(rsum_all[:], rsum_ps[:, N_SROW - 1 : N_SROW])
                    ctx_sb = small_pool.tile(
                        [128, 512], F32, tag="ctx_sb", bufs=2, name=f"ctxsb{b}"
                    )
                    # DVE normalizes the four live 512-wide groups (~200ns
                    # each vs ~770ns per ACT copy)
                    for jj in range(4):
                        nc.vector.tensor_scalar_mul(
                            ctx_sb[32 * jj : 32 * jj + 1, :],
                            ctx_ps[32 * jj : 32 * jj + 1, :],
                            rsum_all[32 * jj : 32 * jj + 1, 0:1],
                        )
                    nc.sync.dma_start(
                        out=out[b : b + 1, :].rearrange("o (jj d) -> (o jj) d", jj=4),
                        in_=ctx_sb[0:128:32, :],
                    )

                return epi

            for b in range(BL):
                batch_state[b] = (
                    et_pool.tile([128, N_SROW], F32, tag="etp", bufs=1, name="etps"),
                    ctx_pool.tile([128, 512], F32, tag="ctx", bufs=1, name="ctxps"),
                    small_pool.tile([128, N_SROW], BF16, tag="eT", bufs=2, name=f"eT{b}"),
                    small_pool.tile([1, N_ST], F32, tag="esum", bufs=2, name=f"esum{b}"),
                )
                for sp in range(N_SP):
                    stA, stB = 2 * sp, 2 * sp + 1
                    first_pair = (b == 0 and sp == 0)
                    last_pair = (b == BL - 1 and sp == N_SP - 1)
                    encT_A = enc_tiles[(b, stA)]
                    encT_B = enc_tiles[(b, stB)]

                    def prefetch_next(b=b, sp=sp):
                        if sp + 1 < N_SP:
                            issue_enct(b, 2 * sp + 2)
                            issue_enct(b, 2 * sp + 3)
                        elif b + 1 < BL:
                            issue_enct(b + 1, 0)
                            issue_enct(b + 1, 1)

                    # natural-layout rows for this pair's pass-2 (consumed by
                    # the tails emitted during pair sp+1); FIFO behind UwT8
                    # on the SWDGE ring staggers the transfers automatically
                    encNs = {}
                    for st in (stA, stB):
                        rows = []
                        for r in range(st * 4, (st + 1) * 4):
                            encN = encn_pool.tile(
                                [128, D], BF16, tag="encN", bufs=16, name="encN"
                            )
                            nc.gpsimd.dma_start(
                                out=encN[:], in_=enc[b][r * 128 : (r + 1) * 128, :]
                            )
                            rows.append(encN)
                        encNs[st] = rows

                    sc_A = sc_pool.tile([128, S_TILE], F32, tag="sc", bufs=2, name="scpsA")
                    sc_B = sc_pool.tile([128, S_TILE], F32, tag="sc", bufs=2, name="scpsB")
                    acts = {}

                    def v_pair(jp, acts=acts, sc_A=sc_A, sc_B=sc_B):
                        act_A, act_B = acts[jp]
                        v_w = V8_s[:, 2 * jp : 2 * jp + 2, 0:1]
                        nc.tensor.matmul(
                            sc_A[0:1, :], v_w, act_A[:],
                            start=(jp == 0), stop=(jp == KP - 1),
                            perf_mode=PM.DoubleRow,
                        )
                        nc.tensor.matmul(
                            sc_B[0:1, :], v_w, act_B[:],
                            start=(jp == 0), stop=(jp == KP - 1),
                            perf_mode=PM.DoubleRow,
                        )

                    for jp in range(KP):
                        act_A = act_pool.tile(
                            [128, 2, S_TILE], FP8, tag="act", bufs=10, name="actA"
                        )
                        act_B = act_pool.tile(
                            [128, 2, S_TILE], FP8, tag="act", bufs=10, name="actB"
                        )
                        acts[jp] = (act_A, act_B)
                        for jj in range(2):
                            j = 2 * jp + jj
                            if b == 0 and sp == 0:
                                # wq chunk j runs BEFORE U group j so tanh j
                                # never gates the uk bank rotation; it must
                                # also precede the first tanh reading chunk j
                                # (Tile's RAW tracking follows trace order)
                                emit_wq(j)
                            uk_A = uk_pool.tile(
                                [128, S_TILE], F32, tag="uk", bufs=4, name="ukpsA"
                            )
                            uk_B = uk_pool.tile(
                                [128, S_TILE], F32, tag="uk", bufs=4, name="ukpsB"
                            )
                            if first_pair:
                                # de-interleaved: A's tile arrives first, so
                                # an interleaved B matmul would head-of-line
                                # block A's stream on the (0,1) tile DMA
                                for t in range(DT2):
                                    nc.tensor.matmul(
                                        uk_A[:],
                                        UwT8_s[:, j, 2 * t : 2 * t + 2, :],
                                        encT_A[:, 2 * t : 2 * t + 2, :],
                                        start=(t == 0), stop=(t == DT2 - 1),
                                        perf_mode=PM.DoubleRow,
                                    )
                                for t in range(DT2):
                                    nc.tensor.matmul(
                                        uk_B[:],
                                        UwT8_s[:, j, 2 * t : 2 * t + 2, :],
                                        encT_B[:, 2 * t : 2 * t + 2, :],
                                        start=(t == 0), stop=(t == DT2 - 1),
                                        perf_mode=PM.DoubleRow,
                                    )
                            else:
                                for t in range(DT2):
                                    w_tj = UwT8_s[:, j, 2 * t : 2 * t + 2, :]
                                    nc.tensor.matmul(
                                        uk_A[:], w_tj, encT_A[:, 2 * t : 2 * t + 2, :],
                                        start=(t == 0), stop=(t == DT2 - 1),
                                        perf_mode=PM.DoubleRow,
                                    )
                                    nc.tensor.matmul(
                                        uk_B[:], w_tj, encT_B[:, 2 * t : 2 * t + 2, :],
                                        start=(t == 0), stop=(t == DT2 - 1),
                                        perf_mode=PM.DoubleRow,
                                    )
                            nc.scalar.activation(
                                act_A[:, jj, :], uk_A[:], AF.Tanh,
                                bias=wqb[:, j * BL + b : j * BL + b + 1],
                                scale=INV_UK,
                            )
                            nc.scalar.activation(
                                act_B[:, jj, :], uk_B[:], AF.Tanh,
                                bias=wqb[:, j * BL + b : j * BL + b + 1],
                                scale=INV_UK,
                            )
                        if jp == 0 and carry_v[0] is not None:
                            # previous pair's V matmuls, batched: one weight-
                            # pipeline break instead of one per j-pair
                            carry_v[0]()
                            carry_v[0] = None
                        if jp == 1:
                            # previous pair's exp/eT/ctx, now safely overlapped
                            emit_pending()
                            prefetch_next()
                        if last_pair and jp > 0:
                            # incremental V on the last pair: only V(3) + the
                            # exps remain after the final U matmuls
                            v_pair(jp - 1)

                    if last_pair:
                        carry_v[0] = lambda: v_pair(KP - 1)
                    else:
                        def v_block(acts=acts, sc_A=sc_A, sc_B=sc_B):
                            for jp in range(KP):
                                v_pair(jp, acts=acts, sc_A=sc_A, sc_B=sc_B)

                        carry_v[0] = v_block

                    pending.append(make_tail(b, stA, sc_A, encNs[stA]))
                    pending.append(make_tail(b, stB, sc_B, encNs[stB]))
                if b == BL - 1:
                    carry_v[0]()
                    carry_v[0] = None
                    emit_pending()
                    make_epilogue(b)()
                else:
                    pending.append(make_epilogue(b))

    _split_sync_waits(nc)
    return nc


_NC_CACHE = None


def _get_nc():
    global _NC_CACHE
    if _NC_CACHE is None:
        _NC_CACHE = build_nc()
    return _NC_CACHE


def _prep_in_maps(encoder_annotations, decoder_prev_hidden, W_w, W_b, U_w, U_b, V_w, V_b):
    enc_f = np.asarray(encoder_annotations, np.float32)
    enc_bf = enc_f.astype(ml_dtypes.bfloat16)
    dh = np.asarray(decoder_prev_hidden, np.float32)[0]      # [B, H]
    W_w = np.asarray(W_w, np.float32)
    U_w = np.asarray(U_w, np.float32)
    V_w = np.asarray(V_w, np.float32)
    bias_sum = (np.asarray(W_b, np.float32) + np.asarray(U_b, np.float32))  # [H]

    # WwT[p, j, i, m] = W_w[j*128+m, i*128+p], bf16 (per-j chunk layout)
    WwT_s = np.ascontiguousarray(
        W_w.T.reshape(HT, 128, KSUB, 128).transpose(1, 2, 0, 3)
    ).astype(ml_dtypes.bfloat16)
    # UwT8[p, j, dt, m] = U_SCALE * U_w[j*128+m, dt*128+p], e4m3
    UwT8_s = np.ascontiguousarray(
        (U_SCALE * U_w).T.reshape(DT, 128, KSUB, 128).transpose(1, 2, 0, 3)
    ).astype(ml_dtypes.float8_e4m3)
    # Vw8[p, j, 0] = V_SCALE * V_w[0, j*128+p], e4m3 (16-col padded planes)
    v8 = (V_SCALE * V_w[0]).reshape(KSUB, 128).T.astype(ml_dtypes.float8_e4m3)
    Vw8_s = np.zeros((128, KSUB, 128), ml_dtypes.float8_e4m3)
    Vw8_s[:, :, 0] = v8
    # bias_ext: cols 0..7 = (W_b + U_b) lanes; col 8 = -M0 with
    # M0 = sum|v8|/V_SCALE >= max|score| since |act| <= 1
    bias_s = np.zeros((128, 512), np.float32)
    bias_s[:, :KSUB] = bias_sum.reshape(KSUB, 128).T
    bias_s[:, 8] = -float(np.abs(v8.astype(np.float32)).sum() / V_SCALE)

    in_maps = []
    for c in range(N_CORES):
        enc_c = enc_f[c * BL : (c + 1) * BL]                 # [BL, S, D] f32
        # encT8[(b st), p, dt, s] = ENC_SCALE * enc[b, st*512+s, dt*128+p]
        enc8 = (ENC_SCALE * enc_c).astype(ml_dtypes.float8_e4m3)
        encT8_c = np.ascontiguousarray(
            enc8.reshape(BL, N_ST, S_TILE, DT, 128).transpose(0, 1, 4, 3, 2)
        ).reshape(BL * N_ST, 128, DT, S_TILE)
        dh_c = dh[c * BL : (c + 1) * BL]                     # [BL, H]
        dhT_c = np.zeros((128, 1024), ml_dtypes.bfloat16)
        dhT_c[:, : HT * BL] = dh_c.T.reshape(HT, 128, BL).transpose(1, 0, 2).reshape(
            128, HT * BL
        ).astype(ml_dtypes.bfloat16)
        in_maps.append(
            {
                "encT8": encT8_c,
                "enc": np.ascontiguousarray(enc_bf[c * BL : (c + 1) * BL]),
                "dhT": dhT_c,
                "WwT": WwT_s,
                "UwT8": UwT8_s,
                "Vw8": Vw8_s,
                "bias": bias_s,
            }
        )
    return in_maps


def run(inputs, trace=False):
    """Run on hardware; returns (full_output, BassKernelResults)."""
    nc = _get_nc()
    in_maps = _prep_in_maps(**inputs)
    res = run_bass_kernel_spmd(nc, in_maps, list(range(N_CORES)), trace=trace)
    ctx = np.concatenate([np.asarray(r["out"], np.float32) for r in res.results], axis=0)
    return ctx.reshape(B, 1, D), res


def kernel(**inputs) -> np.ndarray:
    out, _ = run(inputs, trace=False)
    return out
